# revision 1
# baseline (speedup 1.0000x reference)
"""CogVideoX spatial+temporal attention block on 8 Trainium2 NeuronCores.

Strategy:
  Pass 1 (spatial attention): data-parallel over the 32 frames (B*T), 4 frames
  per core, full attention over the 576 tokens of each frame.
  Pass 2 (causal temporal attention): data-parallel over the 1152 pixels
  (B*HW), 144 pixels per core; per pixel, causal attention over T=16 frames.
  Pixels are batched 8-per-128-token-group with a block-diagonal causal mask.
  The host reshards (transposes) between the passes.

  Each pass is one Bass program per core (SPMD on cores 0-7), all activations
  kept d-major ("x^T": feature dim on partitions) so QKV, scores, AV, and
  proj are all natural matmuls with no on-chip transposes. Softmax
  denominators ride along as a ones-column in the AV matmul (row 64 of the
  PSUM output); the per-query reciprocal is broadcast across partitions with
  a rank-1 PE matmul. Matmuls run in float32r (TF32-like, ~1e-4 rel err,
  4x faster than fp32 on the PE) accumulating in fp32 PSUM.
"""

import contextlib
import ctypes
import sys
import types

sys.path.insert(0, "/opt/trn_rl_repo")

import numpy as np  # noqa: E402

import concourse.bass as bass  # noqa: E402
import concourse.mybir as mybir  # noqa: E402
import concourse.tile as tile  # noqa: E402

F32 = mybir.dt.float32
F32R = mybir.dt.float32r
AF = mybir.ActivationFunctionType

B, T, H, W, D = 2, 16, 24, 24, 1024
HW = H * W            # 576
NH, HD = 16, 64
NCORES = 8
TOK = 2304            # tokens per core, both passes

# ---------------------------------------------------------------------------
# Environment shims for this container
# ---------------------------------------------------------------------------

def _install_env_shims():
    # 1) NTFF profile hook: trn_boot only registers it when the image's
    #    antenv package has an axon_hooks module; fabricate one.
    import antenv
    if "antenv.axon_hooks" not in sys.modules:
        mod = types.ModuleType("antenv.axon_hooks")
        hook_box = [None]
        mod.set_axon_ntff_profile_hook = lambda h: hook_box.__setitem__(0, h)
        mod.get_axon_ntff_profile_hook = lambda: hook_box[0]
        sys.modules["antenv.axon_hooks"] = mod
        antenv.axon_hooks = mod
        try:
            lib = ctypes.CDLL("/opt/axon/libaxon_pjrt.so")
            if hasattr(lib, "axon_start_nrt_profile"):
                lib.axon_start_nrt_profile.argtypes = [
                    ctypes.POINTER(ctypes.c_int64), ctypes.c_size_t]
                lib.axon_start_nrt_profile.restype = ctypes.c_int64
                lib.axon_stop_nrt_profile.argtypes = [ctypes.c_char_p]
                lib.axon_stop_nrt_profile.restype = ctypes.c_int64

                @contextlib.contextmanager
                def _hook(output_dir, device_ids):
                    import jax
                    jax.devices()
                    if device_ids:
                        ids = (ctypes.c_int64 * len(device_ids))(*device_ids)
                        rc = lib.axon_start_nrt_profile(ids, len(device_ids))
                    else:
                        rc = lib.axon_start_nrt_profile(None, 0)
                    if rc != 0:
                        raise RuntimeError(f"axon_start_nrt_profile rc={rc}")
                    try:
                        yield
                    finally:
                        lib.axon_stop_nrt_profile(str(output_dir).encode())

                mod.set_axon_ntff_profile_hook(_hook)
        except OSError:
            pass
    # 2) No bucket access in this container; keep profile artifacts local.
    from concourse import bass_utils
    bass_utils.upload_artifacts = lambda tmpdir: f"local:{tmpdir}"


_install_env_shims()


def _split_sync_waits(nc):
    """This container's walrus build rejects instructions carrying more than
    one sync-wait command; hoist excess waits onto NoOps inserted immediately
    before the instruction on the same engine."""
    n = 0
    for bb in nc.main_func.blocks:
        new_insts = []
        for inst in bb.instructions:
            si = inst.sync_info
            waits = list(si.on_wait) if (si and si.on_wait) else []
            if len(waits) > 1:
                si.on_wait.clear()
                for w in waits[:-1]:
                    nop = mybir.InstNoOp(
                        name=f"{inst.name}-ws{n}",
                        engine=inst.engine,
                        sync_info=mybir.SyncInfo(on_wait=[w], on_update=[]),
                        bass_nofuse=True,
                    )
                    n += 1
                    nc.register_instruction(nop)
                    new_insts.append(nop)
                si.on_wait.append(waits[-1])
            new_insts.append(inst)
        bb.instructions[:] = new_insts
    return n


# ---------------------------------------------------------------------------
# Kernel builder (shared by the spatial and temporal passes)
# ---------------------------------------------------------------------------

def _build_pass(mode):
    assert mode in ("spatial", "temporal")
    temporal = mode == "temporal"
    nc = bass.Bass()

    xt = nc.declare_dram_parameter("xt", [D, TOK], F32R, isOutput=False)
    wqk = nc.declare_dram_parameter("wqk", [D, 2 * D], F32R, isOutput=False)
    wv = nc.declare_dram_parameter("wv", [D, D], F32R, isOutput=False)
    wp = nc.declare_dram_parameter("wp", [64, 16, D], F32R, isOutput=False)
    qkb = nc.declare_dram_parameter("qkb", [128, 16], F32, isOutput=False)
    vb = nc.declare_dram_parameter("vb", [128, D], F32, isOutput=False)
    pb = nc.declare_dram_parameter("pb", [128, 8], F32, isOutput=False)
    mask = None
    if temporal:
        mask = nc.declare_dram_parameter("mask", [128, 256], F32R, isOutput=False)
    out = nc.declare_dram_parameter("out", [D, TOK], F32, isOutput=True)

    # DRAM scratch between phase 1 (QKV) and phase 2 (attention+proj)
    qkt_d = nc.dram_tensor("qkt_d", [2 * D, TOK], F32R)
    v_d = nc.dram_tensor("v_d", [TOK, D], F32R)

    with tile.TileContext(nc) as tc:
        # ------------------------------------------------------------------
        # Phase 1: qkT = (x @ Wqk)^T  (d-major),  V = x @ Wv  (token-major)
        # ------------------------------------------------------------------
        with (
            tc.tile_pool(name="p1s", bufs=1) as pool,
            tc.tile_pool(name="p1p", bufs=1, space="PSUM") as psum,
        ):
            xt_t = pool.tile([128, 8, TOK], F32R, tag="xt", name="xt", bufs=1)
            nc.sync.dma_start(xt_t[:], xt.rearrange("(c p) t -> p c t", p=128))
            qkb_t = pool.tile([128, 16], F32, tag="qkb", name="qkb", bufs=1)
            nc.sync.dma_start(qkb_t[:], qkb[:])
            vb_t = pool.tile([128, D], F32, tag="vb", name="vb", bufs=1)
            nc.sync.dma_start(vb_t[:], vb[:])
            wv_t = pool.tile([128, 8, D], F32R, tag="wv", name="wv", bufs=1)
            nc.sync.dma_start(wv_t[:], wv.rearrange("(c p) m -> p c m", p=128))

            wqk_r = wqk.rearrange("(c p) m -> p c m", p=128)
            NT = [(0, 512), (512, 512), (1024, 512), (1536, 512), (2048, 256)]
            for co in range(16):
                wblk = pool.tile([128, 8, 128], F32R, tag="wqkblk", name="wqkblk", bufs=3)
                nc.sync.dma_start(wblk[:], wqk_r[:, :, co * 128:(co + 1) * 128])
                for (t0, tn) in NT:
                    ps = psum.tile([128, 512], F32, tag="p512", name="p512", bufs=4)[:, :tn]
                    for ci in range(8):
                        nc.tensor.matmul(
                            ps, wblk[:, ci, :], xt_t[:, ci, t0:t0 + tn],
                            start=(ci == 0), stop=(ci == 7))
                    st = pool.tile([128, 512], F32R, tag="qkst", name="qkst", bufs=4)[:, :tn]
                    nc.scalar.activation(st, ps, AF.Identity,
                                         bias=qkb_t[:, co:co + 1])
                    nc.sync.dma_start(
                        qkt_d[co * 128:(co + 1) * 128, t0:t0 + tn], st)

            for mt in range(18):
                for vc in range(2):
                    ps = psum.tile([128, 512], F32, tag="p512", name="p512", bufs=4)
                    for ci in range(8):
                        nc.tensor.matmul(
                            ps, xt_t[:, ci, mt * 128:(mt + 1) * 128],
                            wv_t[:, ci, vc * 512:(vc + 1) * 512],
                            start=(ci == 0), stop=(ci == 7))
                    st = pool.tile([128, 512], F32R, tag="vst", name="vst", bufs=4)
                    nc.vector.tensor_add(st[:], ps[:],
                                         vb_t[:, vc * 512:(vc + 1) * 512])
                    nc.sync.dma_start(
                        v_d[mt * 128:(mt + 1) * 128, vc * 512:(vc + 1) * 512],
                        st[:])

        # ------------------------------------------------------------------
        # Phase 2: per block, attention + proj + residual
        # PSUM tiles are laid out [128, n_qsplits, 512] so every matmul
        # output lands inside a single 2KB PSUM bank.
        # ------------------------------------------------------------------
        if temporal:
            blocks = [(sb * 256, 256) for sb in range(9)]
            QS = [(0, 0, 256)]            # (slot, q0, qn)
            n_kc = 2      # two independent 128-token causal groups
        else:
            blocks = [(f * 576, 576) for f in range(4)]
            QS = [(0, 0, 288), (1, 288, 288)]
            KC = [(0, 128), (128, 128), (256, 128), (384, 128), (512, 64)]
            n_kc = len(KC)
        NS = len(QS)

        with (
            tc.tile_pool(name="p2s", bufs=1) as pool,
            tc.tile_pool(name="p2p", bufs=1, space="PSUM") as psum,
        ):
            pb_t = pool.tile([128, 8], F32, tag="pb", name="pb", bufs=1)
            nc.sync.dma_start(pb_t[:], pb[:])
            onef = pool.tile([128, 64], F32, tag="onef", name="onef", bufs=1)
            nc.any.memset(onef[:], 1.0)
            ones_t = pool.tile([128, 64], F32R, tag="ones", name="ones", bufs=1)
            nc.vector.tensor_copy(ones_t[:], onef[:])
            mask_t = None
            if temporal:
                mask_t = pool.tile([128, 256], F32R, tag="mask", name="mask",
                                   bufs=1)
                nc.sync.dma_start(mask_t[:], mask[:])

            qkt_r = qkt_d.rearrange("(c p) t -> p c t", p=128)
            xt_r = xt.rearrange("(c p) t -> p c t", p=128)

            for (t0, tn) in blocks:
                qkt_f = pool.tile([128, 16, tn], F32R, tag="qktf",
                                  name="qktf", bufs=1)
                nc.sync.dma_start(qkt_f[:], qkt_r[:, :, t0:t0 + tn])
                vx = pool.tile([128, n_kc, 16, 65], F32R, tag="vx", name="vx",
                               bufs=1)
                for ki in range(n_kc):
                    kn = min(128, tn - ki * 128)
                    nc.sync.dma_start(
                        vx[0:kn, ki, :, 0:64],
                        v_d[t0 + ki * 128:t0 + ki * 128 + kn, :]
                        .rearrange("p (h e) -> p h e", e=64))
                # ones column for the fused softmax denominator
                nc.vector.tensor_copy(
                    vx[:, :, :, 64:65],
                    onef[:, 0:1][:, None, None, :].to_broadcast(
                        (128, n_kc, 16, 1)))
                xt_f = pool.tile([128, 8, tn], F32R, tag="xtf", name="xtf",
                                 bufs=1)
                nc.sync.dma_start(xt_f[:], xt_r[:, :, t0:t0 + tn])

                attn = pool.tile([64, 16, tn], F32R, tag="attn", name="attn",
                                 bufs=2)
                for h in range(16):
                    c, lo = h // 2, (h % 2) * 64
                    qT = qkt_f[lo:lo + 64, c, :]
                    kT = qkt_f[lo:lo + 64, 8 + c, :]
                    rc = pool.tile([128, tn], F32R, tag="rc", name="rc",
                                   bufs=2)
                    if temporal:
                        ek = pool.tile([128, tn], F32R, tag="ek", name="ek",
                                       bufs=3)
                        for gi in range(2):
                            g0 = gi * 128
                            sps = psum.tile([128, 1, 512], F32, tag="pp",
                                            name="pp", bufs=4)
                            nc.tensor.matmul(sps[:, 0, 0:256],
                                             kT[:, g0:g0 + 128], qT,
                                             start=True, stop=True)
                            nc.scalar.activation(
                                ek[:, g0:g0 + 128], sps[:, 0, g0:g0 + 128],
                                AF.Exp, scale=0.125)
                        nc.vector.tensor_mul(ek[:], ek[:], mask_t[:])
                        avs = []
                        for gi in range(2):
                            g0 = gi * 128
                            av = psum.tile([128, 1, 512], F32, tag="av",
                                           name="av", bufs=4)
                            nc.tensor.matmul(av[0:65, 0, 0:256],
                                             vx[:, gi, h, :],
                                             ek[:], start=True, stop=True)
                            with nc.allow_low_precision(
                                    reason="f32r recip feeds f32r mm"):
                                nc.vector.reciprocal(
                                    rc[64:65, g0:g0 + 128],
                                    av[64:65, 0, g0:g0 + 128])
                            avs.append(av)
                    else:
                        av = psum.tile([128, NS, 512], F32, tag="av",
                                       name="av", bufs=2)
                        for ki, (k0, kn) in enumerate(KC):
                            sps = psum.tile([128, NS, 512], F32, tag="pp",
                                            name="pp", bufs=2)
                            for (si, q0, qn) in QS:
                                nc.tensor.matmul(
                                    sps[0:kn, si, 0:qn], kT[:, k0:k0 + kn],
                                    qT[:, q0:q0 + qn], start=True, stop=True)
                            ek = pool.tile([128, tn], F32R, tag="ek",
                                           name="ek", bufs=3)
                            for (si, q0, qn) in QS:
                                nc.scalar.activation(
                                    ek[0:kn, q0:q0 + qn], sps[0:kn, si, 0:qn],
                                    AF.Exp, scale=0.125)
                            for (si, q0, qn) in QS:
                                nc.tensor.matmul(
                                    av[0:65, si, 0:qn], vx[0:kn, ki, h, :],
                                    ek[0:kn, q0:q0 + qn],
                                    start=(ki == 0), stop=(ki == n_kc - 1))
                        with nc.allow_low_precision(
                                reason="f32r recip feeds f32r mm"):
                            for (si, q0, qn) in QS:
                                nc.vector.reciprocal(rc[64:65, q0:q0 + qn],
                                                     av[64:65, si, 0:qn])
                    # broadcast 1/denom across the 64 head dims via rank-1 mm
                    bc = psum.tile([128, NS, 512], F32, tag="pp", name="pp",
                                   bufs=4 if temporal else 2)
                    for (si, q0, qn) in QS:
                        nc.tensor.matmul(bc[0:64, si, 0:qn],
                                         ones_t[64:65, 0:64],
                                         rc[64:65, q0:q0 + qn],
                                         start=True, stop=True)
                    bcs = pool.tile([64, tn], F32, tag="bcs", name="bcs",
                                    bufs=2)
                    for (si, q0, qn) in QS:
                        nc.vector.tensor_copy(bcs[:, q0:q0 + qn],
                                              bc[0:64, si, 0:qn])
                    if temporal:
                        for gi in range(2):
                            g0 = gi * 128
                            nc.vector.tensor_mul(
                                attn[:, h, g0:g0 + 128],
                                avs[gi][0:64, 0, g0:g0 + 128],
                                bcs[:, g0:g0 + 128])
                    else:
                        for (si, q0, qn) in QS:
                            nc.vector.tensor_mul(attn[:, h, q0:q0 + qn],
                                                 av[0:64, si, 0:qn],
                                                 bcs[:, q0:q0 + qn])

                for dc in range(8):
                    wpb = pool.tile([64, 16, 128], F32R, tag="wpb",
                                    name="wpb", bufs=2)
                    nc.sync.dma_start(wpb[:],
                                      wp[:, :, dc * 128:(dc + 1) * 128])
                    pps = psum.tile([128, NS, 512], F32, tag="pp", name="pp",
                                    bufs=4 if temporal else 2)
                    for h in range(16):
                        for (si, q0, qn) in QS:
                            nc.tensor.matmul(
                                pps[:, si, 0:qn], wpb[0:64, h, :],
                                attn[0:64, h, q0:q0 + qn],
                                start=(h == 0), stop=(h == 15))
                    osb = pool.tile([128, tn], F32, tag="osb", name="osb",
                                    bufs=3)
                    for (si, q0, qn) in QS:
                        nc.scalar.activation(osb[:, q0:q0 + qn],
                                             pps[:, si, 0:qn], AF.Identity,
                                             bias=pb_t[:, dc:dc + 1])
                    nc.vector.tensor_add(osb[:], osb[:],
                                         xt_f[:, dc, :].bitcast(F32))
                    nc.sync.dma_start(
                        out[dc * 128:(dc + 1) * 128, t0:t0 + tn], osb[:])

    _split_sync_waits(nc)
    return nc


_PROGRAMS = {}


def _get_program(mode):
    if mode not in _PROGRAMS:
        _PROGRAMS[mode] = _build_pass(mode)
    return _PROGRAMS[mode]


# ---------------------------------------------------------------------------
# Host wrapper
# ---------------------------------------------------------------------------

TRACE = False
LAST_EXEC_NS = {}
LAST_PROFILE = {}


def _run_pass(mode, xt_cores, wqkv, bqkv, wproj, bproj, mask01=None):
    from concourse.bass_utils import run_bass_kernel_spmd
    nc = _get_program(mode)
    wqkv = np.asarray(wqkv, np.float32)
    wqk = np.ascontiguousarray(wqkv[:, :2 * D])
    wv = np.ascontiguousarray(wqkv[:, 2 * D:])
    wp_r = np.ascontiguousarray(
        np.asarray(wproj, np.float32).reshape(16, 64, D).transpose(1, 0, 2))
    bqkv = np.asarray(bqkv, np.float32)
    qkb = np.ascontiguousarray(bqkv[:2 * D].reshape(16, 128).T)
    vb_b = np.ascontiguousarray(
        np.broadcast_to(bqkv[2 * D:], (128, D)))
    pb = np.ascontiguousarray(np.asarray(bproj, np.float32).reshape(8, 128).T)
    in_maps = []
    for c in range(NCORES):
        m = {"xt": xt_cores[c], "wqk": wqk, "wv": wv, "wp": wp_r,
             "qkb": qkb, "vb": vb_b, "pb": pb}
        if mask01 is not None:
            m["mask"] = mask01
        in_maps.append(m)
    res = run_bass_kernel_spmd(nc, in_maps, core_ids=list(range(NCORES)),
                               trace=TRACE)
    if TRACE:
        LAST_EXEC_NS[mode] = res.exec_time_ns
        LAST_PROFILE[mode] = res.profile_json
    return [res.results[c]["out"] for c in range(NCORES)]


def kernel(x, ws_qkv, bs_qkv, ws_proj, bs_proj, wt_qkv, bt_qkv, wt_proj,
           bt_proj, T=T, H=H, W=W, **_kw):
    x = np.asarray(x, np.float32)
    # ---- pass 1: spatial, shard over frames --------------------------------
    xT = np.ascontiguousarray(x.reshape(B * T * HW, D).T)   # (D, 18432)
    xt_cores = [np.ascontiguousarray(xT[:, c * 4 * HW:(c + 1) * 4 * HW])
                for c in range(NCORES)]
    outs = _run_pass("spatial", xt_cores, ws_qkv, bs_qkv, ws_proj, bs_proj)
    x1T = np.concatenate(outs, axis=1)                      # (D, 18432)
    # ---- reshard: (D, B, T, HW) -> (D, B, HW, T), shard over pixels --------
    x1p = np.ascontiguousarray(
        x1T.reshape(D, B, T, HW).transpose(0, 1, 3, 2).reshape(D, B * HW * T))
    kk, qq = np.meshgrid(np.arange(128), np.arange(128), indexing="ij")
    mask01 = (((kk // 16) == (qq // 16)) & ((kk % 16) <= (qq % 16))
              ).astype(np.float32)
    mask01 = np.ascontiguousarray(np.tile(mask01, (1, 2)))
    xt_cores2 = [np.ascontiguousarray(x1p[:, c * TOK:(c + 1) * TOK])
                 for c in range(NCORES)]
    outs2 = _run_pass("temporal", xt_cores2, wt_qkv, bt_qkv, wt_proj, bt_proj,
                      mask01)
    x2p = np.concatenate(outs2, axis=1)
    out = x2p.reshape(D, B, HW, T).transpose(0, 1, 3, 2).reshape(D, B * T * HW)
    return np.ascontiguousarray(out.T).reshape(B, T * HW, D)



# revision 27
# speedup vs baseline: 1.1318x; 1.1318x over previous
"""CogVideoX spatial+temporal attention block on 8 Trainium2 NeuronCores.

Strategy:
  Pass 1 (spatial attention): data-parallel over the 32 frames (B*T), 4 frames
  per core, full attention over the 576 tokens of each frame.
  Pass 2 (causal temporal attention): data-parallel over the 1152 pixels
  (B*HW), 144 pixels per core; per pixel, causal attention over T=16 frames.
  Pixels are batched 8-per-128-token-group with a block-diagonal causal mask.
  The host reshards (transposes) between the passes.

  Each pass is one Bass program per core (SPMD on cores 0-7), all activations
  kept d-major ("x^T": feature dim on partitions) so QKV, scores, AV, and
  proj are all natural matmuls with no on-chip transposes. Softmax
  denominators ride along as a ones-column in the AV matmul (row 64 of the
  PSUM output); the per-query reciprocal is broadcast across partitions with
  a rank-1 PE matmul. Matmuls run in float32r (TF32-like, ~1e-4 rel err,
  4x faster than fp32 on the PE) accumulating in fp32 PSUM.
"""

import contextlib
import ctypes
import sys
import types

sys.path.insert(0, "/opt/trn_rl_repo")

import numpy as np  # noqa: E402

import concourse.bass as bass  # noqa: E402
import concourse.mybir as mybir  # noqa: E402
import concourse.tile as tile  # noqa: E402

F32 = mybir.dt.float32
F32R = mybir.dt.float32r
BF = mybir.dt.bfloat16
AF = mybir.ActivationFunctionType

B, T, H, W, D = 2, 16, 24, 24, 1024
HW = H * W            # 576
NH, HD = 16, 64
NCORES = 8
TOK = 2304            # tokens per core, both passes

# ---------------------------------------------------------------------------
# Environment shims for this container
# ---------------------------------------------------------------------------

def _install_env_shims():
    # 1) NTFF profile hook: trn_boot only registers it when the image's
    #    antenv package has an axon_hooks module; fabricate one.
    import antenv
    if "antenv.axon_hooks" not in sys.modules:
        mod = types.ModuleType("antenv.axon_hooks")
        hook_box = [None]
        mod.set_axon_ntff_profile_hook = lambda h: hook_box.__setitem__(0, h)
        mod.get_axon_ntff_profile_hook = lambda: hook_box[0]
        sys.modules["antenv.axon_hooks"] = mod
        antenv.axon_hooks = mod
        try:
            lib = ctypes.CDLL("/opt/axon/libaxon_pjrt.so")
            if hasattr(lib, "axon_start_nrt_profile"):
                lib.axon_start_nrt_profile.argtypes = [
                    ctypes.POINTER(ctypes.c_int64), ctypes.c_size_t]
                lib.axon_start_nrt_profile.restype = ctypes.c_int64
                lib.axon_stop_nrt_profile.argtypes = [ctypes.c_char_p]
                lib.axon_stop_nrt_profile.restype = ctypes.c_int64

                @contextlib.contextmanager
                def _hook(output_dir, device_ids):
                    import jax
                    jax.devices()
                    if device_ids:
                        ids = (ctypes.c_int64 * len(device_ids))(*device_ids)
                        rc = lib.axon_start_nrt_profile(ids, len(device_ids))
                    else:
                        rc = lib.axon_start_nrt_profile(None, 0)
                    if rc != 0:
                        raise RuntimeError(f"axon_start_nrt_profile rc={rc}")
                    try:
                        yield
                    finally:
                        lib.axon_stop_nrt_profile(str(output_dir).encode())

                mod.set_axon_ntff_profile_hook(_hook)
        except OSError:
            pass
    # 2) No bucket access in this container; keep profile artifacts local.
    from concourse import bass_utils
    bass_utils.upload_artifacts = lambda tmpdir: f"local:{tmpdir}"


_install_env_shims()


def _split_sync_waits(nc):
    """This container's walrus build rejects instructions carrying more than
    one sync-wait command; hoist excess waits onto NoOps inserted immediately
    before the instruction on the same engine."""
    n = 0
    for bb in nc.main_func.blocks:
        new_insts = []
        for inst in bb.instructions:
            si = inst.sync_info
            waits = list(si.on_wait) if (si and si.on_wait) else []
            if len(waits) > 1:
                si.on_wait.clear()
                for w in waits[:-1]:
                    nop = mybir.InstNoOp(
                        name=f"{inst.name}-ws{n}",
                        engine=inst.engine,
                        sync_info=mybir.SyncInfo(on_wait=[w], on_update=[]),
                        bass_nofuse=True,
                    )
                    n += 1
                    nc.register_instruction(nop)
                    new_insts.append(nop)
                si.on_wait.append(waits[-1])
            new_insts.append(inst)
        bb.instructions[:] = new_insts
    return n


def _recip_fast(nc, out_ap, scratch_ap, in_ap, use_scalar):
    """Softmax-denominator reciprocal straight to the f32r tile the
    broadcast matmul consumes (custom DVE ops fail codegen on this
    walrus build, so plain InstReciprocal it is)."""
    with nc.allow_low_precision(reason="f32r recip feeds f32r mm"):
        nc.vector.reciprocal(out_ap, in_ap)


# ---------------------------------------------------------------------------
# Kernel builder (shared by the spatial and temporal passes)
# ---------------------------------------------------------------------------

def _build_pass(mode):
    assert mode in ("spatial", "temporal")
    temporal = mode == "temporal"
    nc = bass.Bass()

    xt = nc.declare_dram_parameter("xt", [D, TOK], BF, isOutput=False)
    wqk = nc.declare_dram_parameter("wqk", [D, 2 * D], BF, isOutput=False)
    wv = nc.declare_dram_parameter("wv", [D, D], BF, isOutput=False)
    wp = nc.declare_dram_parameter("wp", [64, 16, D], BF, isOutput=False)
    qkb = nc.declare_dram_parameter("qkb", [128, 16], F32, isOutput=False)
    vb = nc.declare_dram_parameter("vb", [128, D], F32, isOutput=False)
    pb = nc.declare_dram_parameter("pb", [128, 8], F32, isOutput=False)
    mask = None
    if temporal:
        mask = nc.declare_dram_parameter("mask", [128, 256], BF, isOutput=False)
    out = nc.declare_dram_parameter("out", [D, TOK], F32, isOutput=True)

    # DRAM scratch between phase 1 (QKV) and phase 2 (attention+proj)
    qkt_d = nc.dram_tensor("qkt_d", [2 * D, TOK], BF)
    v_d = nc.dram_tensor("v_d", [TOK, D], BF)

    with tile.TileContext(nc) as tc:
        # ------------------------------------------------------------------
        # Phase 1: qkT = (x @ Wqk)^T  (d-major),  V = x @ Wv  (token-major)
        # ------------------------------------------------------------------
        with (
            tc.tile_pool(name="p1s", bufs=1) as pool,
            tc.tile_pool(name="p1p", bufs=1, space="PSUM") as psum,
        ):
            xt_t = pool.tile([128, 8, TOK], BF, tag="xt", name="xt", bufs=1)
            nc.sync.dma_start(xt_t[:], xt.rearrange("(c p) t -> p c t", p=128))
            qkb_t = pool.tile([128, 16], F32, tag="qkb", name="qkb", bufs=1)
            nc.sync.dma_start(qkb_t[:], qkb[:])
            vb_t = pool.tile([128, D], F32, tag="vb", name="vb", bufs=1)
            nc.sync.dma_start(vb_t[:], vb[:])
            wv_t = pool.tile([128, 8, D], BF, tag="wv", name="wv", bufs=1)
            nc.sync.dma_start(wv_t[:], wv.rearrange("(c p) m -> p c m", p=128))

            wqk_r = wqk.rearrange("(c p) m -> p c m", p=128)
            NT = [(0, 512), (512, 512), (1024, 512), (1536, 512), (2048, 256)]
            for co in range(16):
                wblk = pool.tile([128, 8, 128], BF, tag="wqkblk", name="wqkblk", bufs=3)
                nc.sync.dma_start(wblk[:], wqk_r[:, :, co * 128:(co + 1) * 128])
                for (t0, tn) in NT:
                    ps = psum.tile([128, 512], F32, tag="p512", name="p512", bufs=4)[:, :tn]
                    for ci in range(8):
                        nc.tensor.matmul(
                            ps, wblk[:, ci, :], xt_t[:, ci, t0:t0 + tn],
                            start=(ci == 0), stop=(ci == 7))
                    st = pool.tile([128, 512], BF, tag="qkst", name="qkst", bufs=4)[:, :tn]
                    nc.scalar.activation(st, ps, AF.Identity,
                                         bias=qkb_t[:, co:co + 1])
                    nc.sync.dma_start(
                        qkt_d[co * 128:(co + 1) * 128, t0:t0 + tn], st)

            for mt in range(18):
                for vc in range(2):
                    ps = psum.tile([128, 512], F32, tag="p512", name="p512", bufs=4)
                    for ci in range(8):
                        nc.tensor.matmul(
                            ps, xt_t[:, ci, mt * 128:(mt + 1) * 128],
                            wv_t[:, ci, vc * 512:(vc + 1) * 512],
                            start=(ci == 0), stop=(ci == 7))
                    st = pool.tile([128, 512], BF, tag="vst", name="vst", bufs=4)
                    nc.vector.tensor_add(st[:], ps[:],
                                         vb_t[:, vc * 512:(vc + 1) * 512])
                    nc.sync.dma_start(
                        v_d[mt * 128:(mt + 1) * 128, vc * 512:(vc + 1) * 512],
                        st[:])

        # ------------------------------------------------------------------
        # Phase 2: per block, attention + proj + residual
        # PSUM tiles are laid out [128, n_qsplits, 512] so every matmul
        # output lands inside a single 2KB PSUM bank.
        # ------------------------------------------------------------------
        if temporal:
            blocks = [(sb * 256, 256) for sb in range(9)]
            QS = [(0, 0, 256)]            # (slot, q0, qn)
            n_kc = 2      # two independent 128-token causal groups
        else:
            blocks = [(f * 576, 576) for f in range(4)]
            QS = [(0, 0, 288), (1, 288, 288)]
            KC = [(0, 128), (128, 128), (256, 128), (384, 128), (512, 64)]
            n_kc = len(KC)
        NS = len(QS)

        with (
            tc.tile_pool(name="p2s", bufs=1) as pool,
            tc.tile_pool(name="p2p", bufs=1, space="PSUM") as psum,
        ):
            pb_t = pool.tile([128, 8], F32, tag="pb", name="pb", bufs=1)
            nc.sync.dma_start(pb_t[:], pb[:])
            onef = pool.tile([128, 64], F32, tag="onef", name="onef", bufs=1)
            nc.any.memset(onef[:], 1.0)
            ones_t = pool.tile([128, 64], F32R, tag="ones", name="ones", bufs=1)
            nc.vector.tensor_copy(ones_t[:], onef[:])
            oneb = pool.tile([128, 64], BF, tag="oneb", name="oneb", bufs=1)
            nc.vector.tensor_copy(oneb[:], onef[:])
            mask_t = None
            if temporal:
                mask_t = pool.tile([128, 256], BF, tag="mask", name="mask",
                                   bufs=1)
                nc.sync.dma_start(mask_t[:], mask[:])

            qkt_r = qkt_d.rearrange("(c p) t -> p c t", p=128)
            xt_r = xt.rearrange("(c p) t -> p c t", p=128)

            for (t0, tn) in blocks:
                qkt_f = pool.tile([128, 16, tn], BF, tag="qktf",
                                  name="qktf", bufs=1)
                nc.sync.dma_start(qkt_f[:], qkt_r[:, :, t0:t0 + tn])
                vx = pool.tile([128, n_kc, 16, 65], BF, tag="vx", name="vx",
                               bufs=1)
                for ki in range(n_kc):
                    kn = min(128, tn - ki * 128)
                    nc.sync.dma_start(
                        vx[0:kn, ki, :, 0:64],
                        v_d[t0 + ki * 128:t0 + ki * 128 + kn, :]
                        .rearrange("p (h e) -> p h e", e=64))
                # ones column for the fused softmax denominator
                nc.vector.tensor_copy(
                    vx[:, :, :, 64:65],
                    onef[:, 0:1][:, None, None, :].to_broadcast(
                        (128, n_kc, 16, 1)))
                xt_f = pool.tile([128, 8, tn], BF, tag="xtf", name="xtf",
                                 bufs=1)
                nc.sync.dma_start(xt_f[:], xt_r[:, :, t0:t0 + tn])

                attn = pool.tile([64, 16, tn], BF, tag="attn", name="attn",
                                 bufs=2)
                for h in range(16):
                    c, lo = h // 2, (h % 2) * 64
                    qT = qkt_f[lo:lo + 64, c, :]
                    kT = qkt_f[lo:lo + 64, 8 + c, :]
                    rc = pool.tile([128, tn], F32R, tag="rc", name="rc",
                                   bufs=2)
                    rcf = pool.tile([128, tn], F32, tag="rcf", name="rcf",
                                    bufs=2)
                    if temporal:
                        ek = pool.tile([128, tn], BF, tag="ek", name="ek",
                                       bufs=3)
                        for gi in range(2):
                            g0 = gi * 128
                            sps = psum.tile([128, 1, 512], F32, tag="pp",
                                            name="pp", bufs=4)
                            nc.tensor.matmul(sps[:, 0, 0:256],
                                             kT[:, g0:g0 + 128], qT,
                                             start=True, stop=True)
                            nc.scalar.activation(
                                ek[:, g0:g0 + 128], sps[:, 0, g0:g0 + 128],
                                AF.Exp, scale=0.125)
                        nc.vector.tensor_mul(ek[:], ek[:], mask_t[:])
                        avs = []
                        for gi in range(2):
                            g0 = gi * 128
                            av = psum.tile([128, 1, 512], F32, tag="av",
                                           name="av", bufs=4)
                            nc.tensor.matmul(av[0:65, 0, 0:256],
                                             vx[:, gi, h, :],
                                             ek[:], start=True, stop=True)
                            _recip_fast(nc, rc[64:65, g0:g0 + 128],
                                        rcf[64:65, g0:g0 + 128],
                                        av[64:65, 0, g0:g0 + 128], True)
                            avs.append(av)
                    else:
                        av = psum.tile([128, NS, 512], F32, tag="av",
                                       name="av", bufs=2)
                        for ki, (k0, kn) in enumerate(KC):
                            sps = psum.tile([128, NS, 512], F32, tag="pp",
                                            name="pp", bufs=2)
                            for (si, q0, qn) in QS:
                                nc.tensor.matmul(
                                    sps[0:kn, si, 0:qn], kT[:, k0:k0 + kn],
                                    qT[:, q0:q0 + qn], start=True, stop=True)
                            ek = pool.tile([128, tn], BF, tag="ek",
                                           name="ek", bufs=3)
                            for (si, q0, qn) in QS:
                                nc.scalar.activation(
                                    ek[0:kn, q0:q0 + qn], sps[0:kn, si, 0:qn],
                                    AF.Exp, scale=0.125)
                            for (si, q0, qn) in QS:
                                nc.tensor.matmul(
                                    av[0:65, si, 0:qn], vx[0:kn, ki, h, :],
                                    ek[0:kn, q0:q0 + qn],
                                    start=(ki == 0), stop=(ki == n_kc - 1))
                        for (si, q0, qn) in QS:
                            _recip_fast(nc, rc[64:65, q0:q0 + qn],
                                        rcf[64:65, q0:q0 + qn],
                                        av[64:65, si, 0:qn], False)
                    # broadcast 1/denom across the 64 head dims via rank-1 mm
                    bc = psum.tile([128, NS, 512], F32, tag="pp", name="pp",
                                   bufs=4 if temporal else 2)
                    for (si, q0, qn) in QS:
                        nc.tensor.matmul(bc[0:64, si, 0:qn],
                                         ones_t[64:65, 0:64],
                                         rc[64:65, q0:q0 + qn],
                                         start=True, stop=True)
                    bcs = pool.tile([64, tn], F32, tag="bcs", name="bcs",
                                    bufs=2)
                    for (si, q0, qn) in QS:
                        nc.vector.tensor_copy(bcs[:, q0:q0 + qn],
                                              bc[0:64, si, 0:qn])
                    if temporal:
                        for gi in range(2):
                            g0 = gi * 128
                            nc.vector.tensor_mul(
                                attn[:, h, g0:g0 + 128],
                                avs[gi][0:64, 0, g0:g0 + 128],
                                bcs[:, g0:g0 + 128])
                    else:
                        for (si, q0, qn) in QS:
                            nc.vector.tensor_mul(attn[:, h, q0:q0 + qn],
                                                 av[0:64, si, 0:qn],
                                                 bcs[:, q0:q0 + qn])

                for dc in range(8):
                    wpb = pool.tile([64, 16, 128], BF, tag="wpb",
                                    name="wpb", bufs=2)
                    nc.sync.dma_start(wpb[:],
                                      wp[:, :, dc * 128:(dc + 1) * 128])
                    pps = psum.tile([128, NS, 512], F32, tag="pp", name="pp",
                                    bufs=4 if temporal else 2)
                    for h in range(16):
                        for (si, q0, qn) in QS:
                            nc.tensor.matmul(
                                pps[:, si, 0:qn], wpb[0:64, h, :],
                                attn[0:64, h, q0:q0 + qn],
                                start=(h == 0), stop=(h == 15))
                    osb = pool.tile([128, tn], F32, tag="osb", name="osb",
                                    bufs=3)
                    for (si, q0, qn) in QS:
                        nc.scalar.activation(osb[:, q0:q0 + qn],
                                             pps[:, si, 0:qn], AF.Identity,
                                             bias=pb_t[:, dc:dc + 1])
                    nc.vector.tensor_add(osb[:], osb[:], xt_f[:, dc, :])
                    nc.sync.dma_start(
                        out[dc * 128:(dc + 1) * 128, t0:t0 + tn], osb[:])

    _split_sync_waits(nc)
    return nc


_PROGRAMS = {}


def _get_program(mode):
    if mode not in _PROGRAMS:
        _PROGRAMS[mode] = _build_pass(mode)
    return _PROGRAMS[mode]


# ---------------------------------------------------------------------------
# Host wrapper
# ---------------------------------------------------------------------------

TRACE = False
LAST_EXEC_NS = {}
LAST_PROFILE = {}


def _run_pass(mode, xt_cores, wqkv, bqkv, wproj, bproj, mask01=None):
    import ml_dtypes
    from concourse.bass_utils import run_bass_kernel_spmd
    BF_NP = ml_dtypes.bfloat16
    nc = _get_program(mode)
    wqkv = np.asarray(wqkv, np.float32)
    wqk = np.ascontiguousarray(wqkv[:, :2 * D].astype(BF_NP))
    wv = np.ascontiguousarray(wqkv[:, 2 * D:].astype(BF_NP))
    wp_r = np.ascontiguousarray(
        np.asarray(wproj, np.float32).reshape(16, 64, D).transpose(1, 0, 2)
        .astype(BF_NP))
    bqkv = np.asarray(bqkv, np.float32)
    qkb = np.ascontiguousarray(bqkv[:2 * D].reshape(16, 128).T)
    vb_b = np.ascontiguousarray(
        np.broadcast_to(bqkv[2 * D:], (128, D)))
    pb = np.ascontiguousarray(np.asarray(bproj, np.float32).reshape(8, 128).T)
    if mask01 is not None:
        mask01 = np.ascontiguousarray(np.asarray(mask01).astype(BF_NP))
    in_maps = []
    for c in range(NCORES):
        m = {"xt": xt_cores[c], "wqk": wqk, "wv": wv, "wp": wp_r,
             "qkb": qkb, "vb": vb_b, "pb": pb}
        if mask01 is not None:
            m["mask"] = mask01
        in_maps.append(m)
    res = run_bass_kernel_spmd(nc, in_maps, core_ids=list(range(NCORES)),
                               trace=TRACE)
    if TRACE:
        LAST_EXEC_NS[mode] = res.exec_time_ns
        LAST_PROFILE[mode] = res.profile_json
    return [res.results[c]["out"] for c in range(NCORES)]


def kernel(x, ws_qkv, bs_qkv, ws_proj, bs_proj, wt_qkv, bt_qkv, wt_proj,
           bt_proj, T=T, H=H, W=W, **_kw):
    import ml_dtypes
    BF_NP = ml_dtypes.bfloat16
    x = np.asarray(x, np.float32)
    # ---- pass 1: spatial, shard over frames --------------------------------
    xT = np.ascontiguousarray(x.reshape(B * T * HW, D).T)   # (D, 18432)
    xTb = xT.astype(BF_NP)
    xt_cores = [np.ascontiguousarray(xTb[:, c * 4 * HW:(c + 1) * 4 * HW])
                for c in range(NCORES)]
    outs = _run_pass("spatial", xt_cores, ws_qkv, bs_qkv, ws_proj, bs_proj)
    x1T = np.concatenate(outs, axis=1)                      # (D, 18432)
    # ---- reshard: (D, B, T, HW) -> (D, B, HW, T), shard over pixels --------
    x1p = np.ascontiguousarray(
        x1T.reshape(D, B, T, HW).transpose(0, 1, 3, 2).reshape(D, B * HW * T))
    kk, qq = np.meshgrid(np.arange(128), np.arange(128), indexing="ij")
    mask01 = (((kk // 16) == (qq // 16)) & ((kk % 16) <= (qq % 16))
              ).astype(np.float32)
    mask01 = np.ascontiguousarray(np.tile(mask01, (1, 2)))
    x1pb = x1p.astype(BF_NP)
    xt_cores2 = [np.ascontiguousarray(x1pb[:, c * TOK:(c + 1) * TOK])
                 for c in range(NCORES)]
    outs2 = _run_pass("temporal", xt_cores2, wt_qkv, bt_qkv, wt_proj, bt_proj,
                      mask01)
    x2p = np.concatenate(outs2, axis=1)
    out = x2p.reshape(D, B, HW, T).transpose(0, 1, 3, 2).reshape(D, B * T * HW)
    return np.ascontiguousarray(out.T).reshape(B, T * HW, D)

